# revision 30
# baseline (speedup 1.0000x reference)
"""Trainium2 Bass kernel for nn_Decoder (dense transformer decoder, 2 layers).

Sharding (8 cores): core c = 2*b + r handles batch b, query-row half r.

Structure (v2 — collective-latency oriented rework of the baseline):
- Per layer the only collectives are: {AllReduce(stats1) || AllGather(raw x1
  halves)}, AllReduce(stats2), {AllReduce(stats3) || AllGather(raw res3)}.
  The raw-halves AllGather ships pre-BN bf16 data so it can fly CONCURRENTLY
  with the stats AllReduce instead of serially after it; BN is applied
  locally to the gathered halves once stats land (the V2 AllGather and the
  layer-boundary xin AllGather of the baseline are gone).
- V2 is computed for ALL tokens locally from the gathered t (costs +18K PE
  rows/layer, removes a serial collective from the critical path).
- Collective windows are covered with independent work: cross-attn scores
  (L1) / DRAM e-prefetch (L2).
- Eviction engine balance: score eviction chunks split Act(Square) / DVE
  affine + Pool square so the PE stays the pacer; V2 eviction on DVE;
  BN applies + bf16 staging casts on Pool (GpSimd).
- Self-attention exp(x) ~= (1 + x/2)^2 (Square); softmax denominators via
  V-aug ones column; per-head reciprocal broadcast through a K=1 matmul.
- Cross-attention scores/exp depend only on `encod`: computed in L1, cached
  in DRAM, streamed back in L2. Cross reciprocals cached too.
- bv2 / bo2 / bf biases dropped (the following train-mode BN cancels
  constant shifts exactly). bv / bq / bk / bq2 / bk2 stay.
"""
import numpy as np
import ml_dtypes

B, S, D, H = 4, 1024, 768, 12
HD = D // H          # 64
R = S // 2           # 512 own rows per core
NC = 8
NLAYERS = 2
SCALE1 = 1.0 / float(np.sqrt(D))
SCALE2 = 1.0 / float(np.sqrt(HD))
INV_N = 1.0 / (B * S)

_CACHE = {}


def _pos_encoding():
    p = np.arange(S, dtype=np.float32)[:, None]
    i = np.arange(D // 2, dtype=np.float32)[None, :]
    ang = p / np.power(10000.0, 2.0 * i / D)
    return np.stack([np.sin(ang), np.cos(ang)], axis=-1).reshape(S, D).astype(np.float32)


def _fm(a):
    """[tok, feat] -> feature-major chunked [128, nchunk, tok]."""
    t, f = a.shape
    return np.ascontiguousarray(a.T.reshape(f // 128, 128, t).transpose(1, 0, 2))


def _wchunk(w):
    """[in, out] weight -> [128, nin, out] (stationary chunks)."""
    i, o = w.shape
    return np.ascontiguousarray(w.reshape(i // 128, 128, o).transpose(1, 0, 2))


def _col(v):
    """[768] -> [128, 6] feature-major columns."""
    return np.ascontiguousarray(v.reshape(6, 128).T)


def _bf16(a):
    return np.asarray(a, np.float32).astype(ml_dtypes.bfloat16)


def _build(layers=NLAYERS, stage=99):
    import concourse.bass as bass
    import concourse.mybir as mybir
    import concourse.tile as tile
    from concourse import bacc

    BF = mybir.dt.bfloat16
    F32 = mybir.dt.float32
    AF = mybir.ActivationFunctionType
    OP = mybir.AluOpType

    nc = bacc.Bacc(None, target_bir_lowering=False, debug=False)

    # ---- I/O (identical contract to the baseline) ----
    xin_io = nc.dram_tensor("xin", [128, 6, S], BF, kind="ExternalInput")
    xq_io = nc.dram_tensor("xq", [128, 2, R], BF, kind="ExternalInput")
    xo_io = nc.dram_tensor("xo", [128, 6, R], F32, kind="ExternalInput")
    encq_io = nc.dram_tensor("encq", [128, 3, R], BF, kind="ExternalInput")
    enck_io = nc.dram_tensor("enck", [128, 3, S], BF, kind="ExternalInput")
    w_io = {}
    for nm, nin in [("wq", 2), ("wk", 2), ("wv", 2), ("wq2", 3), ("wk2", 3),
                    ("wv2", 6), ("wo2", 6), ("wf", 6)]:
        w_io[nm] = nc.dram_tensor(nm, [128, nin, D], BF, kind="ExternalInput")
    # cvec cols: bq 0-5, bk 6-11, bq2 12-17, bk2 18-23, g1 24-29, b1 30-35,
    #            g2 36-41, b2 42-47
    cvec_io = nc.dram_tensor("cvec", [128, 48], F32, kind="ExternalInput")
    brow_io = nc.dram_tensor("brow", [1, D], BF, kind="ExternalInput")  # bv
    out_io = nc.dram_tensor("out", [128, 6, R], BF, kind="ExternalOutput")

    PAIRS = [[0, 1], [2, 3], [4, 5], [6, 7]]
    ALL8 = [list(range(NC))]

    with tile.TileContext(nc) as tc:
        with (
            tc.tile_pool(name="pp", bufs=1) as pp,
            tc.tile_pool(name="trans", bufs=1) as tr,
            tc.tile_pool(name="resp", bufs=3) as resp,
            tc.tile_pool(name="epool", bufs=2) as epool,
            tc.tile_pool(name="sqp", bufs=1) as sqp,
            tc.tile_pool(name="smallp", bufs=1) as smallp,
            tc.tile_pool(name="ps_sc", bufs=2, space="PSUM") as ps_sc,
            tc.tile_pool(name="ps_av", bufs=2, space="PSUM") as ps_av,
            tc.tile_pool(name="ps_g", bufs=2, space="PSUM") as ps_g,
            tc.tile_pool(name="dram", bufs=1, space="DRAM") as dram,
        ):
            # ---- persistent SBUF loads, ordered by first use (input DMA
            # sustains only ~140GB/s, so order is critical: Q's inputs, then
            # K/V's, then the cross-attn preamble set, then bulk weights) ----
            w_sb = {}

            def wload(nm):
                t_io = w_io[nm]
                w_sb[nm] = pp.tile(list(t_io.shape), BF, name=f"sb_{nm}")
                nc.sync.dma_start(w_sb[nm][:], t_io[:])

            wload("wq")
            cvec = pp.tile([128, 48], F32, name="sb_cvec")
            nc.sync.dma_start(cvec[:], cvec_io[:])
            xq1 = tr.tile([128, 2, R], BF, tag="xq", bufs=1)
            nc.sync.dma_start(xq1[:], xq_io[:])
            # warm-up AllReduce: after the Q-critical loads so they win
            # the DMA queue, but early enough to warm the ring
            warm_sb = pp.tile([128, 12], F32, name="sb_warm")
            nc.vector.memset(warm_sb[:], 0.0)
            war_in = dram.tile([128, 12], F32, tag="arwarm")
            war_out = dram.tile([128, 12], F32, tag="arwarmo",
                                addr_space="Shared")
            nc.sync.dma_start(war_in[:], warm_sb[:])
            nc.gpsimd.collective_compute(
                "AllReduce", OP.add, replica_groups=ALL8,
                ins=[war_in[:].opt()], outs=[war_out[:].opt()])
            nc.sync.dma_start(warm_sb[:], war_out[:])
            wload("wk")
            xin = pp.tile([128, 6, S], BF, name="sb_xin")
            nc.sync.dma_start(xin[:], xin_io[:])
            wload("wv")
            bias_v = pp.tile([128, D], BF, name="sb_biasv")
            nc.sync.dma_start(out=bias_v[:, :],
                              in_=brow_io[0:1, :].broadcast_to([128, D]))
            xo1 = resp.tile([128, 6, R], F32, tag="res", name="sb_xo1")
            nc.sync.dma_start(xo1[:], xo_io[:])
            # encq/enck share the "xg" rotation with the x1 gather staging
            # tiles (same byte size; they die after the cross preamble).
            encq_t = tr.tile([128, 6, R], BF, tag="xg", bufs=2)
            encq = encq_t[:, 0:3, :]
            nc.sync.dma_start(encq, encq_io[:])
            enck_t = tr.tile([128, 6, R], BF, tag="xg", bufs=2)
            enck = enck_t.rearrange("p (a b) r -> p a (b r)", a=3)
            nc.sync.dma_start(enck[:], enck_io[:])
            wload("wq2")
            wload("wk2")
            wload("wv2")
            wload("wo2")
            wload("wf")

            # second warmup AllReduce, gated on the last input load: acts
            # as a loose cross-core barrier absorbing DMA-phase skew so the
            # first real collectives run on an aligned, warm channel.
            warm2_sb = pp.tile([128, 12], F32, name="sb_warm2")
            nc.vector.tensor_copy(warm2_sb[:], w_sb["wk2"][0:128, 0, 0:12])
            war2_in = dram.tile([128, 12], F32, tag="arwarm2")
            war2_out = dram.tile([128, 12], F32, tag="arwarm2o",
                                 addr_space="Shared")
            nc.sync.dma_start(war2_in[:], warm2_sb[:])
            nc.gpsimd.collective_compute(
                "AllReduce", OP.add, replica_groups=ALL8,
                ins=[war2_in[:].opt()], outs=[war2_out[:].opt()])

            zero_col = pp.tile([128, 1], F32, name="sb_zero")
            nc.vector.memset(zero_col[:], 0.0)
            one_col = pp.tile([128, 1], F32, name="sb_one")
            nc.vector.memset(one_col[:], 1.0)
            eps_col = pp.tile([128, 1], F32, name="sb_eps")
            nc.vector.memset(eps_col[:], 1e-5)
            ones_m = pp.tile([1, 128], BF, name="sb_onesm")
            nc.vector.memset(ones_m[:], 1.0)

            s1sav = pp.tile([128, 6], F32, name="sb_s1sav")
            s2sav = pp.tile([128, 6], F32, name="sb_s2sav")

            # saved cross-attention reciprocals (bf16), reused in L2
            rcpd = dram.tile([1, H, R], BF, tag="rcpd")
            # cross-attention exp'd scores stored for layer 2
            a2d = dram.tile([128, 8, H * 512], BF, tag="a2d")

            # ---- helpers ----
            def dense_R(w, nin, rhs_fn, evict_fn):
                """R-column dense: out^T[128j+p, q]; psum from ps_g."""
                for j in range(6):
                    ps = ps_g.tile([128, 512], F32, tag="pg")
                    for i in range(nin):
                        nc.tensor.matmul(
                            ps[:, 0:R],
                            w[:, i, j * 128:(j + 1) * 128],
                            rhs_fn(i),
                            start=(i == 0), stop=(i == nin - 1))
                    evict_fn(j, ps)

            def dense_S(w, nin, rhs_fn, evict_fn):
                """S-column dense: both 512-chunks of a j share one
                [128,2,512] psum tile (stationary reuse across chunks)."""
                for j in range(6):
                    ps = ps_sc.tile([128, 2, 512], F32, tag="psc")
                    for i in range(nin):
                        for ci in range(2):
                            nc.tensor.matmul(
                                ps[:, ci, :],
                                w[:, i, j * 128:(j + 1) * 128],
                                rhs_fn(i, ci * 512, 512),
                                start=(i == 0), stop=(i == nin - 1))
                    for ci in range(2):
                        evict_fn(j, ci * 512, ps[:, ci, :])

            def vtok(w, x_lhs_fn, ntok, dst, relu):
                """V / V2 production: token-major [tok, 12*65] with ones col.
                dst [128, ntok//128, 780]. relu eviction on Act; plain
                eviction (V2) on DVE so the Act queue stays clear for exp."""
                ntch = ntok // 128
                for tch in range(ntch):
                    nc.vector.memset(
                        dst[:, tch, :].rearrange("p (h k) -> p h k", k=65)[:, :, 64:65],
                        1.0)
                    for half in range(2):
                        ps = ps_g.tile([128, 512], F32, tag="pg")
                        nin = w.shape[1]
                        for i in range(nin):
                            nc.tensor.matmul(
                                ps[:, 0:384],
                                x_lhs_fn(i, tch),
                                w[:, i, half * 384:(half + 1) * 384],
                                start=(i == 0), stop=(i == nin - 1))
                        dstap = dst[:, tch, :].rearrange(
                            "p (h k) -> p h k", k=65)[:, half * 6:(half + 1) * 6, 0:64]
                        src = ps[:, 0:384].rearrange("p (h k) -> p h k", k=64)
                        if relu:
                            nc.vector.tensor_tensor(
                                ps[:, 0:384], ps[:, 0:384],
                                bias_v[:, half * 384:(half + 1) * 384], op=OP.add)
                            nc.scalar.activation(dstap, src, AF.Relu,
                                                 bias=zero_col[:])
                        else:
                            nc.vector.tensor_copy(dstap, src)

            def attn_head_scores(h, qt_ap, kt_ap_fn, e, mode, scale):
                """Scores + e for head h into e [128, 8, 512]. For the
                square path, chunk p=3 is evicted via DVE affine + Pool
                multiply to offload the Act engine."""
                for p in range(4):
                    sc = ps_sc.tile([128, 2, 512], F32, tag="psc")
                    for t in range(2):
                        j = 2 * p + t
                        nc.tensor.matmul(
                            sc[:, t, 0:R],
                            kt_ap_fn(j),
                            qt_ap,
                            start=True, stop=True)
                    if mode == "square":
                        if p < 3:
                            nc.scalar.activation(e[:, 2 * p:2 * p + 2, :],
                                                 sc[:, :, 0:R], AF.Square,
                                                 bias=one_col[:],
                                                 scale=scale * 0.5)
                        else:
                            y = sqp.tile([128, 2, 512], BF, tag="ysq", bufs=1)
                            nc.vector.tensor_scalar(
                                y[:, :, :], sc[:, :, 0:R], scale * 0.5, 1.0,
                                op0=OP.mult, op1=OP.add)
                            nc.gpsimd.tensor_tensor(
                                e[:, 2 * p:2 * p + 2, :], y[:, :, :],
                                y[:, :, :], op=OP.mult)
                    else:
                        nc.scalar.activation(e[:, 2 * p:2 * p + 2, :],
                                             sc[:, :, 0:R], AF.Exp,
                                             bias=zero_col[:], scale=scale)

            def attn_head_av(h, v_t, e):
                """AV for head h; returns po [65, 512] (row 64 = den)."""
                po = ps_av.tile([65, 512], F32, tag="po")
                for j in range(8):
                    nc.tensor.matmul(
                        po[:, 0:R],
                        v_t[:, j, h * 65:h * 65 + 65],
                        e[:, j, :],
                        start=(j == 0), stop=(j == 7))
                return po

            def attn_head_den(po, rcp_pair, parity):
                """den -> rcp -> bf16 cast for one head."""
                den = smallp.tile([1, 1, R], F32, tag="den", bufs=1)
                nc.scalar.copy(den[0:1, 0, :], po[64:65, 0:R])
                rf = smallp.tile([1, 1, R], F32, tag="rcpf", bufs=1)
                nc.vector.reciprocal_approx_fast(rf[0:1, 0, :], den[0:1, 0, :])
                nc.vector.tensor_copy(rcp_pair[0:1, parity, :], rf[0:1, 0, :])

            def attn_pair_finish(jh, poA, poB, rcp_pair, out_fn):
                """PE broadcast of reciprocals -> normalized eviction for
                heads 2jh (poA) and 2jh+1 (poB). rcp_pair: [1,2,R] bf16."""
                bc = ps_g.tile([128, 512], F32, tag="pg")
                nc.tensor.matmul(bc[0:64, 0:R], ones_m[0:1, 0:64],
                                 rcp_pair[0:1, 0, :], start=True, stop=True)
                nc.tensor.matmul(bc[64:128, 0:R], ones_m[0:1, 0:64],
                                 rcp_pair[0:1, 1, :], start=True, stop=True)
                # DVE cannot read two PSUM operands; stage bc in SBUF
                bcs = sqp.tile([128, 512], BF, tag="bcs", bufs=1)
                nc.scalar.copy(bcs[:, 0:R], bc[:, 0:R])
                out_fn(jh, poA, poB, bcs)

            bn_idx = [0]

            def bn_stats_chunk(res, stats_ab, jh):
                """rowsum (DVE) + square-rowsum (Act Square w/ accum).
                stats_ab = (a, b): chunks 0-2 in a, 3-5 in b; each [128,6]
                holds sums in cols 0-2 and square-sums in cols 3-5."""
                t = stats_ab[jh // 3]
                c = jh % 3
                nc.vector.reduce_sum(t[:, c:c + 1], res[:, jh, :],
                                     axis=mybir.AxisListType.X)
                sq = sqp.tile([128, 512], BF, tag="sq")
                nc.scalar.activation(sq[:, 0:R], res[:, jh, :], AF.Square,
                                     bias=zero_col[:],
                                     accum_out=t[:, 3 + c:4 + c])

            def bn_start(stats):
                """One AllReduce over the [128,12] stats tile."""
                i = bn_idx[0]
                bn_idx[0] += 1
                arin = dram.tile([128, 12], F32, tag=f"arin{i}")
                arout = dram.tile([128, 12], F32, tag=f"arout{i}",
                                  addr_space="Shared")
                nc.sync.dma_start(arin[:], stats[:])
                nc.gpsimd.collective_compute(
                    "AllReduce", OP.add, replica_groups=ALL8,
                    ins=[arin[:].opt()], outs=[arout[:].opt()])
                return arout

            def bn_w_half(arout, h, w, gbase, bbase):
                """Finalize one stats half into w cols [18+3h:21+3h] (scale)
                and [24+3h:27+3h] (shift)."""
                g = smallp.tile([128, 6], F32, tag="gstats", bufs=4,
                                name="gst")
                nc.sync.dma_start(g[:], arout[:, 6 * h:6 * h + 6])
                s0 = 3 * h
                mu = w[:, 0 + s0:3 + s0]
                var = w[:, 6 + s0:9 + s0]
                sc = w[:, 18 + s0:21 + s0]
                sh = w[:, 24 + s0:27 + s0]
                nc.vector.tensor_scalar_mul(mu, g[:, 0:3], INV_N)
                nc.vector.tensor_scalar_mul(var, g[:, 3:6], INV_N)
                nc.vector.tensor_tensor(w[:, 12 + s0:15 + s0], mu, mu,
                                        op=OP.mult)
                nc.vector.tensor_tensor(var, var, w[:, 12 + s0:15 + s0],
                                        op=OP.subtract)
                nc.scalar.activation(w[:, 12 + s0:15 + s0], var, AF.Sqrt,
                                     bias=eps_col[:])
                nc.vector.reciprocal_approx_fast(var, w[:, 12 + s0:15 + s0])
                nc.vector.tensor_tensor(sc, var,
                                        cvec[:, gbase + s0:gbase + s0 + 3],
                                        op=OP.mult)
                nc.vector.tensor_tensor(sh, mu, sc, op=OP.mult)
                nc.vector.tensor_tensor(sh,
                                        cvec[:, bbase + s0:bbase + s0 + 3],
                                        sh, op=OP.subtract)
                return w

            def wscale(wt, cols):
                """wt[:, i, :] *= cols[:, i] in place; split across engines."""
                nin = wt.shape[1]
                for i in range(nin):
                    ap = wt[:, i, :]
                    sc = cols[:, i:i + 1]
                    if i % 3 == 0:
                        nc.vector.tensor_scalar(ap, ap, sc, zero_col[:],
                                                op0=OP.mult, op1=OP.add)
                    elif i % 3 == 1:
                        nc.scalar.activation(ap, ap, AF.Identity,
                                             bias=zero_col[:], scale=sc)
                    else:
                        nc.gpsimd.tensor_scalar(ap, ap, sc, zero_col[:],
                                                op0=OP.mult, op1=OP.add)

            def pool_apply(dst, src, w, jw, nch=1):
                """dst = src*scale + shift on GpSimd (SBUF only)."""
                nc.gpsimd.tensor_scalar(dst, src,
                                        w[:, 18 + jw:19 + jw],
                                        w[:, 24 + jw:25 + jw],
                                        op0=OP.mult, op1=OP.add)

            # ================= layers =================
            xo_cur = xo1
            xq_cur = xq1
            res_final = None
            for layer in range(layers):
                first = layer == 0
                last = layer == layers - 1
                # ---- Q/K/V projections ----
                qt = tr.tile([128, 6, R], BF, tag="q6R", bufs=1)
                kt = tr.tile([128, 6, S], BF, tag="k6S", bufs=1)
                dense_R(w_sb["wq"], 2, lambda i: xq_cur[:, i, :],
                        lambda j, ps: nc.scalar.activation(
                            qt[:, j, :], ps[:, 0:R], AF.Relu,
                            bias=cvec[:, 0 + j:1 + j]))
                dense_S(w_sb["wk"], 2, lambda i, c0, cw: xin[:, 2 + i, c0:c0 + cw],
                        lambda j, c0, ps: nc.scalar.activation(
                            kt[:, j, c0:c0 + 512], ps[:, 0:512], AF.Relu,
                            bias=cvec[:, 6 + j:7 + j]))
                vt = tr.tile([128, 8, 780], BF, tag="v780", bufs=1)
                vtok(w_sb["wv"],
                     lambda i, tch: xin[:, 4 + i, tch * 128:(tch + 1) * 128],
                     S, vt, relu=True)
                if first:
                    # cross-attn Q2/K2: emitted after L1 QKV so the PE can
                    # start on Q immediately (enc/wq2/wk2 loads are later in
                    # the DMA order than wq/xq/xin).
                    q2 = tr.tile([128, 6, R], BF, tag="q2", bufs=1)
                    k2 = tr.tile([128, 6, S], BF, tag="k2", bufs=1)
                    dense_R(w_sb["wq2"], 3, lambda i: encq[:, i, :],
                            lambda j, ps: nc.vector.tensor_scalar(
                                q2[:, j, :], ps[:, 0:R],
                                cvec[:, 12 + j:13 + j], None, op0=OP.add))
                    dense_S(w_sb["wk2"], 3,
                            lambda i, c0, cw: enck[:, i, c0:c0 + cw],
                            lambda j, c0, ps: nc.vector.tensor_scalar(
                                k2[:, j, c0:c0 + 512], ps[:, 0:512],
                                cvec[:, 18 + j:19 + j], None, op0=OP.add))

                # ---- self attention -> res (x1 = norm(AV) + xo), stats,
                #      raw-x1 bf16 staging chunks for the pair AllGather ----
                res = resp.tile([128, 6, R], F32, tag="res")
                stats1 = smallp.tile([128, 12], F32, tag=f"st{layer}a")
                stats_a = stats1[:, 0:6]
                stats_b = stats1[:, 6:12]
                w1 = smallp.tile([128, 30], F32, tag="bnw", bufs=2,
                                 name="w1")
                tbf = tr.tile([128, 6, S], BF, tag="tbfS", bufs=1)
                x1g = tr.tile([128, 6, R], BF, tag="xg", bufs=2)
                GSZ = (4, 1, 1)
                GOF = (0, 4, 5)
                ag1in = [dram.tile([128, GSZ[g], R], BF,
                                   tag=f"ag1i{layer}g{g}",
                                   name=f"ag1i{g}") for g in range(3)]
                ag1out = [dram.tile([2, 128, GSZ[g], R], BF,
                                    tag=f"ag1o{layer}g{g}",
                                    name=f"ag1o{g}") for g in range(3)]

                ar1box = []

                def self_out(jh, poA, poB, bc, res=res,
                             stats_ab=(stats_a, stats_b), stats1=stats1,
                             x1g=x1g, ag1in=ag1in, ag1out=ag1out, tbf=tbf,
                             w1=w1, ar1box=ar1box):
                    nc.vector.tensor_tensor(res[0:64, jh, :], poA[0:64, 0:R],
                                            bc[0:64, 0:R], op=OP.mult)
                    nc.vector.tensor_tensor(res[64:128, jh, :], poB[0:64, 0:R],
                                            bc[64:128, 0:R], op=OP.mult)
                    nc.vector.tensor_tensor(res[:, jh, :], res[:, jh, :],
                                            xo_cur[:, jh, :], op=OP.add)
                    bn_stats_chunk(res, stats_ab, jh)
                    if jh == 2:
                        # first stats half -> DRAM early (hidden)
                        pass
                    if jh == 5:
                        # AllReduce first on the serial CC channel: its
                        # result gates everything, the last gather only V2.
                        ar1box.append(bn_start(stats1))
                    g = 0 if jh < 4 else (1 if jh < 5 else 2)
                    nc.scalar.copy(x1g[:, jh, :], res[:, jh, :])
                    nc.sync.dma_start(ag1in[g][:, jh - GOF[g], :],
                                      x1g[:, jh, :])
                    if jh - GOF[g] == GSZ[g] - 1:
                        nc.gpsimd.collective_compute(
                            "AllGather", OP.bypass, replica_groups=PAIRS,
                            ins=[ag1in[g][:].opt()], outs=[ag1out[g][:].opt()])
                        nc.sync.dma_start(
                            tbf[:, GOF[g]:GOF[g] + GSZ[g], 0:R],
                            ag1out[g][0, :, :, :])
                        nc.sync.dma_start(
                            tbf[:, GOF[g]:GOF[g] + GSZ[g], R:S],
                            ag1out[g][1, :, :, :])

                po_pair = [None, None]
                rcp_s = None
                for h in range(H):
                    e = epool.tile([128, 8, 512], BF, tag="e8", bufs=3)
                    attn_head_scores(
                        h, qt[64 * (h % 2):64 * (h % 2) + 64, h // 2, :],
                        lambda j, h=h: kt[64 * (h % 2):64 * (h % 2) + 64,
                                          h // 2, j * 128:(j + 1) * 128],
                        e, "square", SCALE1)
                    po_pair[h % 2] = attn_head_av(h, vt, e)
                    if h % 2 == 0:
                        rcp_s = smallp.tile([1, 2, R], BF, tag="rcps", bufs=2)
                    attn_head_den(po_pair[h % 2], rcp_s, h % 2)
                    if h % 2 == 1:
                        attn_pair_finish(h // 2, po_pair[0], po_pair[1],
                                         rcp_s[0:1, :, :], self_out)

                arout1 = ar1box[0]
                bn_w_half(arout1, 0, w1, 24, 30)
                bn_w_half(arout1, 1, w1, 24, 30)
                if stage <= 1:
                    res_final = res
                    break

                # ---- window filler: first cross-score heads (L1) /
                #      e + rcp prefetch from DRAM (L2) ----
                e_held = {}

                def cross_e(h):
                    e = epool.tile([128, 8, 512], BF, tag="e8", bufs=3)
                    if first:
                        attn_head_scores(
                            h, q2[64 * (h % 2):64 * (h % 2) + 64, h // 2, :],
                            lambda j, h=h: k2[64 * (h % 2):64 * (h % 2) + 64,
                                              h // 2, j * 128:(j + 1) * 128],
                            e, "exp", SCALE2)
                        nc.scalar.dma_start(a2d[:, :, h * 512:(h + 1) * 512],
                                            e[:])
                    else:
                        nc.scalar.dma_start(e[:],
                                            a2d[:, :, h * 512:(h + 1) * 512])
                    return e

                NPRE = 3
                for h in range(NPRE):
                    e_held[h] = cross_e(h)

                # fold BN1's scale into Wv2 (diag(s1)·Wv2): V2 then runs on
                # RAW gathered x1 — no BN applies on the critical path. The
                # BN shift contributes a constant row to V2 that the next
                # train-mode BN cancels exactly. L2 rescales by the ratio
                # s1_L2/s1_L1 (wv2 is modified in place).
                if first:
                    nc.vector.tensor_copy(s1sav[:], w1[:, 18:24])
                    wv2cols = w1[:, 18:24]
                else:
                    rat1 = smallp.tile([128, 6], F32, tag="rat", bufs=2,
                                       name="rat1")
                    nc.vector.reciprocal_approx_fast(rat1[:], s1sav[:])
                    nc.vector.tensor_tensor(rat1[:], rat1[:], w1[:, 18:24],
                                            op=OP.mult)
                    wv2cols = rat1[:]
                wscale(w_sb["wv2"], wv2cols)
                if stage <= 2:
                    res_final = res
                    break

                # ---- V2 for ALL tokens (no V2 collective) ----
                v2 = tr.tile([128, 8, 780], BF, tag="v780", bufs=1)
                vtok(w_sb["wv2"],
                     lambda i, tch: tbf[:, i, tch * 128:(tch + 1) * 128],
                     S, v2, relu=False)
                # t residual (f32, own rows) for x2 = m2@Wo2 + t
                for j in range(6):
                    pool_apply(res[:, j, :], res[:, j, :], w1, j)

                # ---- cross attention AV (+ remaining scores/loads) -> m2 ----
                m2 = tr.tile([128, 6, R], BF, tag="q6R", bufs=1)

                def cross_out(jh, poA, poB, bc, m2=m2):
                    nc.vector.tensor_tensor(m2[0:64, jh, :], poA[0:64, 0:R],
                                            bc[0:64, 0:R], op=OP.mult)
                    nc.vector.tensor_tensor(m2[64:128, jh, :], poB[0:64, 0:R],
                                            bc[64:128, 0:R], op=OP.mult)

                if not last:
                    # gather m2 chunk-groups during cross-AV: Wo2 is then
                    # recomputed for BOTH halves locally (x2 = m2g@Wo2 + t),
                    # so BN2 and BN3 expose only their AllReduce.
                    m2g_t = tr.tile([128, 8, 780], BF, tag="v780", bufs=1)
                    m2g = m2g_t.rearrange("p a b -> p (a b)")[:, 0:6 * S]
                    m2g = m2g.rearrange("p (c s) -> p c s", c=6)
                    agm_in = [dram.tile([128, GSZ[g], R], BF,
                                        tag=f"agm{layer}g{g}",
                                        name=f"agmi{g}") for g in range(3)]
                    agm_out = [dram.tile([2, 128, GSZ[g], R], BF,
                                         tag=f"agmo{layer}g{g}",
                                         name=f"agmo{g}") for g in range(3)]

                po_pair = [None, None]
                rcp_p = None
                for h in range(H):
                    e = e_held.pop(h) if h in e_held else cross_e(h)
                    if h + NPRE < H:
                        e_held[h + NPRE] = cross_e(h + NPRE)
                    po_pair[h % 2] = attn_head_av(h, v2, e)
                    if h % 2 == 0:
                        rcp_p = smallp.tile([1, 2, R], BF, tag="rcps",
                                            bufs=2, name="rcp_p")
                        if not first:
                            nc.sync.dma_start(rcp_p[:],
                                              rcpd[0:1, h:h + 2, :])
                    if first:
                        attn_head_den(po_pair[h % 2], rcp_p, h % 2)
                    if h % 2 == 1:
                        attn_pair_finish(h // 2, po_pair[0], po_pair[1],
                                         rcp_p[0:1, :, :], cross_out)
                        if first:
                            nc.sync.dma_start(rcpd[0:1, h - 1:h + 1, :],
                                              rcp_p[:])
                        if not last:
                            jh = h // 2
                            g = 0 if jh < 4 else (1 if jh < 5 else 2)
                            nc.sync.dma_start(agm_in[g][:, jh - GOF[g], :],
                                              m2[:, jh, :])
                            if jh - GOF[g] == GSZ[g] - 1:
                                nc.gpsimd.collective_compute(
                                    "AllGather", OP.bypass,
                                    replica_groups=PAIRS,
                                    ins=[agm_in[g][:].opt()],
                                    outs=[agm_out[g][:].opt()])
                                nc.sync.dma_start(
                                    m2g[:, GOF[g]:GOF[g] + GSZ[g], 0:R],
                                    agm_out[g][0, :, :, :])
                                nc.sync.dma_start(
                                    m2g[:, GOF[g]:GOF[g] + GSZ[g], R:S],
                                    agm_out[g][1, :, :, :])
                if stage <= 3:
                    res_final = res
                    break

                # ---- x2 = m2 @ Wo2 + t ; stats2 (bo2 dropped: BN removes) ----
                res2 = resp.tile([128, 6, R], F32, tag="res")
                stats2 = smallp.tile([128, 12], F32, tag=f"st{layer}b")
                st2a = stats2[:, 0:6]
                st2b = stats2[:, 6:12]
                w2 = smallp.tile([128, 30], F32, tag="bnw", bufs=2,
                                 name="w2")
                t_prev = res
                ar2box = []

                x2bf = tr.tile([128, 6, R], BF, tag="q2", bufs=1,
                               name="x2bf")

                def wo2_evict(j, ps, res2=res2, stats_ab=(st2a, st2b),
                              t_prev=t_prev, w2=w2, stats2=stats2,
                              ar2box=ar2box, x2bf=x2bf):
                    nc.vector.tensor_tensor(res2[:, j, :], ps[:, 0:R],
                                            t_prev[:, j, :], op=OP.add)
                    bn_stats_chunk(res2, stats_ab, j)
                    nc.scalar.copy(x2bf[:, j, :], res2[:, j, :])
                    if j == 5:
                        ar2box.append(bn_start(stats2))

                dense_R(w_sb["wo2"], 6, lambda i: m2[:, i, :], wo2_evict)
                if not last:
                    # both-halves Wo2 from gathered m2; evict x2 = ps + t
                    # (tbf) into t2g raw — PE work that hides the AR2 wait.
                    t2g = tr.tile([128, 6, S], BF, tag="k2", bufs=1,
                                  name="t2g")
                    for j in range(6):
                        ps = ps_sc.tile([128, 2, 512], F32, tag="psc")
                        for i in range(6):
                            for ci in range(2):
                                nc.tensor.matmul(
                                    ps[:, ci, :],
                                    w_sb["wo2"][:, i, j * 128:(j + 1) * 128],
                                    m2g[:, i, ci * 512:ci * 512 + 512],
                                    start=(i == 0), stop=(i == 5))
                        for ci in range(2):
                            nc.vector.affine_then_add(
                                t2g[:, j, ci * 512:ci * 512 + 512],
                                tbf[:, j, ci * 512:ci * 512 + 512],
                                ps[:, ci, :],
                                w1[:, 18 + j:19 + j], w1[:, 24 + j:25 + j])
                arout2 = ar2box[0]
                bn_w_half(arout2, 0, w2, 36, 42)
                bn_w_half(arout2, 1, w2, 36, 42)   # g2, b2
                # t2 bf16 (FFN moving operand) on Act; res2 in-place f32
                # (x3 residual) on Pool.
                # fold BN2's scale into Wf: the FFN matmul consumes the
                # raw x2 (cast during Wo2); constant shifts cancel in BN3.
                if first:
                    nc.vector.tensor_copy(s2sav[:], w2[:, 18:24])
                    wfcols = w2[:, 18:24]
                else:
                    rat2 = smallp.tile([128, 6], F32, tag="rat", bufs=2,
                                       name="rat2")
                    nc.vector.reciprocal_approx_fast(rat2[:], s2sav[:])
                    nc.vector.tensor_tensor(rat2[:], rat2[:], w2[:, 18:24],
                                            op=OP.mult)
                    wfcols = rat2[:]
                wscale(w_sb["wf"], wfcols)
                for j in range(6):
                    pool_apply(res2[:, j, :], res2[:, j, :], w2, j)
                if stage <= 4:
                    res_final = res2
                    break

                # ---- FFN: x3 = t2 @ Wf + t2 ; stats3 (bf dropped) ----
                res3 = resp.tile([128, 6, R], F32, tag="res")
                stats3 = smallp.tile([128, 12], F32, tag=f"st{layer}c")
                st3a = stats3[:, 0:6]
                st3b = stats3[:, 6:12]
                w3 = smallp.tile([128, 30], F32, tag="bnw", bufs=2,
                                 name="w3")
                ar3box = []

                def wf_evict(j, ps, res3=res3, stats_ab=(st3a, st3b),
                             stats3=stats3, res2=res2, w3=w3, ar3box=ar3box):
                    nc.vector.tensor_tensor(res3[:, j, :], ps[:, 0:R],
                                            res2[:, j, :], op=OP.add)
                    bn_stats_chunk(res3, stats_ab, j)
                    if j == 5:
                        ar3box.append(bn_start(stats3))

                dense_R(w_sb["wf"], 6, lambda i: x2bf[:, i, :], wf_evict)
                if not last:
                    # recompute the FFN for ALL tokens from gathered t2 for
                    # the xin chunks L2 actually reads (K: 2,3 / V: 4,5) —
                    # this PE work fills the AR3 window.
                    for j in range(2, 6):
                        ps = ps_sc.tile([128, 2, 512], F32, tag="psc")
                        for i in range(6):
                            for ci in range(2):
                                nc.tensor.matmul(
                                    ps[:, ci, :],
                                    w_sb["wf"][:, i, j * 128:(j + 1) * 128],
                                    t2g[:, i, ci * 512:ci * 512 + 512],
                                    start=(i == 0), stop=(i == 5))
                        for ci in range(2):
                            nc.vector.affine_then_add(
                                xin[:, j, ci * 512:ci * 512 + 512],
                                t2g[:, j, ci * 512:ci * 512 + 512],
                                ps[:, ci, :],
                                w2[:, 18 + j:19 + j], w2[:, 24 + j:25 + j])
                arout3 = ar3box[0]
                bn_w_half(arout3, 0, w3, 36, 42)
                bn_w_half(arout3, 1, w3, 36, 42)   # g2, b2 (FFN BN)

                if not last:
                    # local-first: xq (Q-L2's input) is derivable from res3
                    # alone, so Q can run while the gathers land. res3 in
                    # place f32 -> xo (Pool); xq bf16 via Act.
                    xq2 = tr.tile([128, 2, R], BF, tag="xq", bufs=1)
                    for j in range(2):
                        nc.vector.tensor_scalar(
                            xq2[:, j, :], res3[:, j, :],
                            w3[:, 18 + j:19 + j], w3[:, 24 + j:25 + j],
                            op0=OP.mult, op1=OP.add)
                    for j in range(2, 6):
                        for half in range(2):
                            ap = xin[:, j, half * R:(half + 1) * R]
                            eng = (2 * j + half) % 3
                            if eng == 0:
                                pool_apply(ap, ap, w3, j)
                            elif eng == 1:
                                nc.scalar.activation(
                                    ap, ap, AF.Identity,
                                    bias=w3[:, 24 + j:25 + j],
                                    scale=w3[:, 18 + j:19 + j])
                            else:
                                nc.vector.tensor_scalar(
                                    ap, ap, w3[:, 18 + j:19 + j],
                                    w3[:, 24 + j:25 + j],
                                    op0=OP.mult, op1=OP.add)
                    for j in range(6):
                        pool_apply(res3[:, j, :], res3[:, j, :], w3, j)
                    xo_cur = res3
                    xq_cur = xq2
                else:
                    # final: BN apply (bf16 out staging) + chunk DMAs split
                    # over the two HWDGE queues
                    obf = tr.tile([128, 6, R], BF, tag="xg", bufs=2,
                                  name="obf")
                    for j in range(6):
                        if j % 3 == 0:
                            nc.gpsimd.tensor_scalar(
                                obf[:, j, :], res3[:, j, :],
                                w3[:, 18 + j:19 + j], w3[:, 24 + j:25 + j],
                                op0=OP.mult, op1=OP.add)
                        elif j % 3 == 1:
                            nc.scalar.activation(obf[:, j, :], res3[:, j, :],
                                                 AF.Identity,
                                                 bias=w3[:, 24 + j:25 + j],
                                                 scale=w3[:, 18 + j:19 + j])
                        else:
                            nc.vector.tensor_scalar(
                                obf[:, j, :], res3[:, j, :],
                                w3[:, 18 + j:19 + j], w3[:, 24 + j:25 + j],
                                op0=OP.mult, op1=OP.add)
                        eng = nc.sync if j % 2 == 0 else nc.scalar
                        eng.dma_start(out_io[:, j, :], obf[:, j, :])
                    res_final = None

            if res_final is not None:
                nc.sync.dma_start(out_io[:], res_final[:])

    nc.compile()
    return nc


def _host_prepare(inputs):
    x = np.asarray(inputs["x"])
    encod = np.asarray(inputs["encod"], np.float32)
    embed = np.asarray(inputs["embed"], np.float32)
    emb = embed[x.astype(np.int64)]
    im0 = 2.0 * emb + _pos_encoding()[None]  # [B,S,D] f32

    wq, wk, wv = (np.asarray(inputs[k], np.float32) for k in ("Wq", "Wk", "Wv"))
    wq2, wk2 = (np.asarray(inputs[k], np.float32) for k in ("Wq2", "Wk2"))
    wv2, wo2, wf = (np.asarray(inputs[k], np.float32) for k in ("Wv2", "Wo2", "Wf"))
    w_np = {nm: _bf16(_wchunk(w)) for nm, w in
            [("wq", wq), ("wk", wk), ("wv", wv), ("wq2", wq2), ("wk2", wk2),
             ("wv2", wv2), ("wo2", wo2), ("wf", wf)]}
    cvec = np.concatenate(
        [_col(np.asarray(inputs[k], np.float32)) for k in
         ("bq", "bk", "bq2", "bk2", "g1", "b1", "g2", "b2")],
        axis=1).astype(np.float32)
    brow = _bf16(np.asarray(inputs["bv"], np.float32)[None, :])

    in_maps = []
    for c in range(NC):
        b_, r_ = c // 2, c % 2
        rows = slice(r_ * R, (r_ + 1) * R)
        m = dict(w_np)
        m["cvec"] = cvec
        m["brow"] = brow
        m["xin"] = _bf16(_fm(im0[b_]))
        m["xq"] = _bf16(_fm(im0[b_][rows, 0:256]))
        m["xo"] = _fm(im0[b_][rows]).astype(np.float32)
        m["encq"] = _bf16(_fm(encod[b_][rows, 0:384]))
        m["enck"] = _bf16(_fm(encod[b_][:, 384:768]))
        in_maps.append(m)
    return in_maps


def _gather(results):
    out = np.zeros((B, S, D), np.float32)
    for c in range(NC):
        b_, r_ = c // 2, c % 2
        a = np.asarray(results[c]["out"], np.float32)  # [128, 6, R] bf16
        out[b_, r_ * R:(r_ + 1) * R] = a.transpose(1, 0, 2).reshape(D, R).T
    return out


def kernel(**inputs) -> np.ndarray:
    from concourse.bass_utils import run_bass_kernel_spmd

    if "nc" not in _CACHE:
        _CACHE["nc"] = _build()
    nc = _CACHE["nc"]
    in_maps = _host_prepare(inputs)
    res = run_bass_kernel_spmd(nc, in_maps, core_ids=list(range(NC)))
    return _gather(res.results)


# revision 31
# speedup vs baseline: 1.0219x; 1.0219x over previous
"""Trainium2 Bass kernel for nn_Decoder (dense transformer decoder, 2 layers).

Sharding (8 cores): core c = 2*b + r handles batch b, query-row half r.

Structure (v2 — collective-latency oriented rework of the baseline):
- Per layer the only collectives are: {AllReduce(stats1) || AllGather(raw x1
  halves)}, AllReduce(stats2), {AllReduce(stats3) || AllGather(raw res3)}.
  The raw-halves AllGather ships pre-BN bf16 data so it can fly CONCURRENTLY
  with the stats AllReduce instead of serially after it; BN is applied
  locally to the gathered halves once stats land (the V2 AllGather and the
  layer-boundary xin AllGather of the baseline are gone).
- V2 is computed for ALL tokens locally from the gathered t (costs +18K PE
  rows/layer, removes a serial collective from the critical path).
- Collective windows are covered with independent work: cross-attn scores
  (L1) / DRAM e-prefetch (L2).
- Eviction engine balance: score eviction chunks split Act(Square) / DVE
  affine + Pool square so the PE stays the pacer; V2 eviction on DVE;
  BN applies + bf16 staging casts on Pool (GpSimd).
- Self-attention exp(x) ~= (1 + x/2)^2 (Square); softmax denominators via
  V-aug ones column; per-head reciprocal broadcast through a K=1 matmul.
- Cross-attention scores/exp depend only on `encod`: computed in L1, cached
  in DRAM, streamed back in L2. Cross reciprocals cached too.
- bv2 / bo2 / bf biases dropped (the following train-mode BN cancels
  constant shifts exactly). bv / bq / bk / bq2 / bk2 stay.
"""
import numpy as np
import ml_dtypes

B, S, D, H = 4, 1024, 768, 12
HD = D // H          # 64
R = S // 2           # 512 own rows per core
NC = 8
NLAYERS = 2
SCALE1 = 1.0 / float(np.sqrt(D))
SCALE2 = 1.0 / float(np.sqrt(HD))
INV_N = 1.0 / (B * S)

_CACHE = {}


def _pos_encoding():
    p = np.arange(S, dtype=np.float32)[:, None]
    i = np.arange(D // 2, dtype=np.float32)[None, :]
    ang = p / np.power(10000.0, 2.0 * i / D)
    return np.stack([np.sin(ang), np.cos(ang)], axis=-1).reshape(S, D).astype(np.float32)


def _fm(a):
    """[tok, feat] -> feature-major chunked [128, nchunk, tok]."""
    t, f = a.shape
    return np.ascontiguousarray(a.T.reshape(f // 128, 128, t).transpose(1, 0, 2))


def _wchunk(w):
    """[in, out] weight -> [128, nin, out] (stationary chunks)."""
    i, o = w.shape
    return np.ascontiguousarray(w.reshape(i // 128, 128, o).transpose(1, 0, 2))


def _col(v):
    """[768] -> [128, 6] feature-major columns."""
    return np.ascontiguousarray(v.reshape(6, 128).T)


def _bf16(a):
    return np.asarray(a, np.float32).astype(ml_dtypes.bfloat16)


def _build(layers=NLAYERS, stage=99):
    import concourse.bass as bass
    import concourse.mybir as mybir
    import concourse.tile as tile
    from concourse import bacc

    BF = mybir.dt.bfloat16
    F32 = mybir.dt.float32
    AF = mybir.ActivationFunctionType
    OP = mybir.AluOpType

    nc = bacc.Bacc(None, target_bir_lowering=False, debug=False)

    # ---- I/O (identical contract to the baseline) ----
    xin_io = nc.dram_tensor("xin", [128, 6, S], BF, kind="ExternalInput")
    xq_io = nc.dram_tensor("xq", [128, 2, R], BF, kind="ExternalInput")
    xo_io = nc.dram_tensor("xo", [128, 6, R], F32, kind="ExternalInput")
    encq_io = nc.dram_tensor("encq", [128, 3, R], BF, kind="ExternalInput")
    enck_io = nc.dram_tensor("enck", [128, 3, S], BF, kind="ExternalInput")
    w_io = {}
    for nm, nin in [("wq", 2), ("wk", 2), ("wv", 2), ("wq2", 3), ("wk2", 3),
                    ("wv2", 6), ("wo2", 6), ("wf", 6)]:
        w_io[nm] = nc.dram_tensor(nm, [128, nin, D], BF, kind="ExternalInput")
    # cvec cols: bq 0-5, bk 6-11, bq2 12-17, bk2 18-23, g1 24-29, b1 30-35,
    #            g2 36-41, b2 42-47
    cvec_io = nc.dram_tensor("cvec", [128, 48], F32, kind="ExternalInput")
    brow_io = nc.dram_tensor("brow", [1, D], BF, kind="ExternalInput")  # bv
    out_io = nc.dram_tensor("out", [128, 6, R], BF, kind="ExternalOutput")

    PAIRS = [[0, 1], [2, 3], [4, 5], [6, 7]]
    ALL8 = [list(range(NC))]

    with tile.TileContext(nc) as tc:
        with (
            tc.tile_pool(name="pp", bufs=1) as pp,
            tc.tile_pool(name="trans", bufs=1) as tr,
            tc.tile_pool(name="resp", bufs=3) as resp,
            tc.tile_pool(name="epool", bufs=2) as epool,
            tc.tile_pool(name="sqp", bufs=1) as sqp,
            tc.tile_pool(name="smallp", bufs=1) as smallp,
            tc.tile_pool(name="ps_sc", bufs=2, space="PSUM") as ps_sc,
            tc.tile_pool(name="ps_av", bufs=2, space="PSUM") as ps_av,
            tc.tile_pool(name="ps_g", bufs=2, space="PSUM") as ps_g,
            tc.tile_pool(name="dram", bufs=1, space="DRAM") as dram,
        ):
            # ---- persistent SBUF loads, ordered by first use (input DMA
            # sustains only ~140GB/s, so order is critical: Q's inputs, then
            # K/V's, then the cross-attn preamble set, then bulk weights) ----
            w_sb = {}

            def wload(nm):
                t_io = w_io[nm]
                w_sb[nm] = pp.tile(list(t_io.shape), BF, name=f"sb_{nm}")
                nc.sync.dma_start(w_sb[nm][:], t_io[:])

            wload("wq")
            cvec = pp.tile([128, 48], F32, name="sb_cvec")
            nc.sync.dma_start(cvec[:], cvec_io[:])
            xq1 = tr.tile([128, 2, R], BF, tag="xq", bufs=1)
            nc.sync.dma_start(xq1[:], xq_io[:])
            # warm-up AllReduce: after the Q-critical loads so they win
            # the DMA queue, but early enough to warm the ring
            warm_sb = pp.tile([128, 12], F32, name="sb_warm")
            nc.vector.memset(warm_sb[:], 0.0)
            war_in = dram.tile([128, 12], F32, tag="arwarm")
            war_out = dram.tile([128, 12], F32, tag="arwarmo",
                                addr_space="Shared")
            nc.sync.dma_start(war_in[:], warm_sb[:])
            nc.gpsimd.collective_compute(
                "AllReduce", OP.add, replica_groups=ALL8,
                ins=[war_in[:].opt()], outs=[war_out[:].opt()])
            nc.sync.dma_start(warm_sb[:], war_out[:])
            wload("wk")
            xin = pp.tile([128, 6, S], BF, name="sb_xin")
            nc.sync.dma_start(xin[:], xin_io[:])
            wload("wv")
            bias_v = pp.tile([128, D], BF, name="sb_biasv")
            nc.sync.dma_start(out=bias_v[:, :],
                              in_=brow_io[0:1, :].broadcast_to([128, D]))
            xo1 = resp.tile([128, 6, R], F32, tag="res", name="sb_xo1")
            nc.sync.dma_start(xo1[:], xo_io[:])
            # encq/enck share the "xg" rotation with the x1 gather staging
            # tiles (same byte size; they die after the cross preamble).
            encq_t = tr.tile([128, 6, R], BF, tag="xg", bufs=2)
            encq = encq_t[:, 0:3, :]
            nc.sync.dma_start(encq, encq_io[:])
            enck_t = tr.tile([128, 6, R], BF, tag="xg", bufs=2)
            enck = enck_t.rearrange("p (a b) r -> p a (b r)", a=3)
            nc.sync.dma_start(enck[:], enck_io[:])
            wload("wq2")
            wload("wk2")
            wload("wv2")
            wload("wo2")
            wload("wf")

            # second warmup AllReduce, gated on the last input load: acts
            # as a loose cross-core barrier absorbing DMA-phase skew so the
            # first real collectives run on an aligned, warm channel.
            warm2_sb = pp.tile([128, 12], F32, name="sb_warm2")
            nc.vector.tensor_copy(warm2_sb[:], w_sb["wk2"][0:128, 0, 0:12])
            war2_in = dram.tile([128, 12], F32, tag="arwarm2")
            war2_out = dram.tile([128, 12], F32, tag="arwarm2o",
                                 addr_space="Shared")
            nc.sync.dma_start(war2_in[:], warm2_sb[:])
            nc.gpsimd.collective_compute(
                "AllReduce", OP.add, replica_groups=ALL8,
                ins=[war2_in[:].opt()], outs=[war2_out[:].opt()])

            zero_col = pp.tile([128, 1], F32, name="sb_zero")
            nc.vector.memset(zero_col[:], 0.0)
            one_col = pp.tile([128, 1], F32, name="sb_one")
            nc.vector.memset(one_col[:], 1.0)
            eps_col = pp.tile([128, 1], F32, name="sb_eps")
            nc.vector.memset(eps_col[:], 1e-5)
            ones_m = pp.tile([1, 128], BF, name="sb_onesm")
            nc.vector.memset(ones_m[:], 1.0)

            s1sav = pp.tile([128, 6], F32, name="sb_s1sav")
            s2sav = pp.tile([128, 6], F32, name="sb_s2sav")

            # saved cross-attention reciprocals (bf16), reused in L2
            rcpd = dram.tile([1, H, R], BF, tag="rcpd")
            # cross-attention exp'd scores stored for layer 2
            a2d = dram.tile([128, 8, H * 512], BF, tag="a2d")

            # ---- helpers ----
            def dense_R(w, nin, rhs_fn, evict_fn):
                """R-column dense: out^T[128j+p, q]; psum from ps_g."""
                for j in range(6):
                    ps = ps_g.tile([128, 512], F32, tag="pg")
                    for i in range(nin):
                        nc.tensor.matmul(
                            ps[:, 0:R],
                            w[:, i, j * 128:(j + 1) * 128],
                            rhs_fn(i),
                            start=(i == 0), stop=(i == nin - 1))
                    evict_fn(j, ps)

            def dense_S(w, nin, rhs_fn, evict_fn):
                """S-column dense: both 512-chunks of a j share one
                [128,2,512] psum tile (stationary reuse across chunks)."""
                for j in range(6):
                    ps = ps_sc.tile([128, 2, 512], F32, tag="psc")
                    for i in range(nin):
                        for ci in range(2):
                            nc.tensor.matmul(
                                ps[:, ci, :],
                                w[:, i, j * 128:(j + 1) * 128],
                                rhs_fn(i, ci * 512, 512),
                                start=(i == 0), stop=(i == nin - 1))
                    for ci in range(2):
                        evict_fn(j, ci * 512, ps[:, ci, :])

            def vtok(w, x_lhs_fn, ntok, dst, relu):
                """V / V2 production: token-major [tok, 12*65] with ones col.
                dst [128, ntok//128, 780]. relu eviction on Act; plain
                eviction (V2) on DVE so the Act queue stays clear for exp."""
                ntch = ntok // 128
                for tch in range(ntch):
                    nc.vector.memset(
                        dst[:, tch, :].rearrange("p (h k) -> p h k", k=65)[:, :, 64:65],
                        1.0)
                    for half in range(2):
                        ps = ps_g.tile([128, 512], F32, tag="pg")
                        nin = w.shape[1]
                        for i in range(nin):
                            nc.tensor.matmul(
                                ps[:, 0:384],
                                x_lhs_fn(i, tch),
                                w[:, i, half * 384:(half + 1) * 384],
                                start=(i == 0), stop=(i == nin - 1))
                        dstap = dst[:, tch, :].rearrange(
                            "p (h k) -> p h k", k=65)[:, half * 6:(half + 1) * 6, 0:64]
                        src = ps[:, 0:384].rearrange("p (h k) -> p h k", k=64)
                        if relu:
                            nc.vector.tensor_tensor(
                                ps[:, 0:384], ps[:, 0:384],
                                bias_v[:, half * 384:(half + 1) * 384], op=OP.add)
                            nc.scalar.activation(dstap, src, AF.Relu,
                                                 bias=zero_col[:])
                        else:
                            nc.vector.tensor_copy(dstap, src)

            def attn_head_scores(h, qt_ap, kt_ap_fn, e, mode, scale):
                """Scores + e for head h into e [128, 8, 512]. For the
                square path, chunk p=3 is evicted via DVE affine + Pool
                multiply to offload the Act engine."""
                for p in range(4):
                    sc = ps_sc.tile([128, 2, 512], F32, tag="psc")
                    for t in range(2):
                        j = 2 * p + t
                        nc.tensor.matmul(
                            sc[:, t, 0:R],
                            kt_ap_fn(j),
                            qt_ap,
                            start=True, stop=True)
                    if mode == "square":
                        if p < 3:
                            nc.scalar.activation(e[:, 2 * p:2 * p + 2, :],
                                                 sc[:, :, 0:R], AF.Square,
                                                 bias=one_col[:],
                                                 scale=scale * 0.5)
                        else:
                            y = sqp.tile([128, 2, 512], BF, tag="ysq", bufs=1)
                            nc.vector.tensor_scalar(
                                y[:, :, :], sc[:, :, 0:R], scale * 0.5, 1.0,
                                op0=OP.mult, op1=OP.add)
                            nc.gpsimd.tensor_tensor(
                                e[:, 2 * p:2 * p + 2, :], y[:, :, :],
                                y[:, :, :], op=OP.mult)
                    else:
                        nc.scalar.activation(e[:, 2 * p:2 * p + 2, :],
                                             sc[:, :, 0:R], AF.Exp,
                                             bias=zero_col[:], scale=scale)

            def attn_head_av(h, v_t, e):
                """AV for head h; returns po [65, 512] (row 64 = den)."""
                po = ps_av.tile([65, 512], F32, tag="po")
                for j in range(8):
                    nc.tensor.matmul(
                        po[:, 0:R],
                        v_t[:, j, h * 65:h * 65 + 65],
                        e[:, j, :],
                        start=(j == 0), stop=(j == 7))
                return po

            def attn_head_den(po, rcp_pair, parity):
                """den -> rcp -> bf16 cast for one head."""
                den = smallp.tile([1, 1, R], F32, tag="den", bufs=1)
                nc.scalar.copy(den[0:1, 0, :], po[64:65, 0:R])
                rf = smallp.tile([1, 1, R], F32, tag="rcpf", bufs=1)
                nc.vector.reciprocal_approx_fast(rf[0:1, 0, :], den[0:1, 0, :])
                nc.vector.tensor_copy(rcp_pair[0:1, parity, :], rf[0:1, 0, :])

            def attn_pair_finish(jh, poA, poB, rcp_pair, out_fn):
                """PE broadcast of reciprocals -> normalized eviction for
                heads 2jh (poA) and 2jh+1 (poB). rcp_pair: [1,2,R] bf16."""
                bc = ps_g.tile([128, 512], F32, tag="pg")
                nc.tensor.matmul(bc[0:64, 0:R], ones_m[0:1, 0:64],
                                 rcp_pair[0:1, 0, :], start=True, stop=True)
                nc.tensor.matmul(bc[64:128, 0:R], ones_m[0:1, 0:64],
                                 rcp_pair[0:1, 1, :], start=True, stop=True)
                # DVE cannot read two PSUM operands; stage bc in SBUF
                bcs = sqp.tile([128, 512], BF, tag="bcs", bufs=1)
                nc.scalar.copy(bcs[:, 0:R], bc[:, 0:R])
                out_fn(jh, poA, poB, bcs)

            bn_idx = [0]

            def bn_stats_chunk(res, stats_ab, jh):
                """rowsum (DVE) + square-rowsum (Act Square w/ accum).
                stats_ab = (a, b): chunks 0-2 in a, 3-5 in b; each [128,6]
                holds sums in cols 0-2 and square-sums in cols 3-5."""
                t = stats_ab[jh // 3]
                c = jh % 3
                nc.vector.reduce_sum(t[:, c:c + 1], res[:, jh, :],
                                     axis=mybir.AxisListType.X)
                sq = sqp.tile([128, 512], BF, tag="sq")
                nc.scalar.activation(sq[:, 0:R], res[:, jh, :], AF.Square,
                                     bias=zero_col[:],
                                     accum_out=t[:, 3 + c:4 + c])

            def bn_start(stats):
                """One AllReduce over the [128,12] stats tile."""
                i = bn_idx[0]
                bn_idx[0] += 1
                arin = dram.tile([128, 12], F32, tag=f"arin{i}")
                arout = dram.tile([128, 12], F32, tag=f"arout{i}",
                                  addr_space="Shared")
                nc.sync.dma_start(arin[:], stats[:])
                nc.gpsimd.collective_compute(
                    "AllReduce", OP.add, replica_groups=ALL8,
                    ins=[arin[:].opt()], outs=[arout[:].opt()])
                return arout

            def bn_w_half(arout, h, w, gbase, bbase):
                """Finalize one stats half into w cols [18+3h:21+3h] (scale)
                and [24+3h:27+3h] (shift)."""
                g = smallp.tile([128, 6], F32, tag="gstats", bufs=4,
                                name="gst")
                nc.sync.dma_start(g[:], arout[:, 6 * h:6 * h + 6])
                s0 = 3 * h
                mu = w[:, 0 + s0:3 + s0]
                var = w[:, 6 + s0:9 + s0]
                sc = w[:, 18 + s0:21 + s0]
                sh = w[:, 24 + s0:27 + s0]
                nc.vector.tensor_scalar_mul(mu, g[:, 0:3], INV_N)
                nc.vector.tensor_scalar_mul(var, g[:, 3:6], INV_N)
                nc.vector.tensor_tensor(w[:, 12 + s0:15 + s0], mu, mu,
                                        op=OP.mult)
                nc.vector.tensor_tensor(var, var, w[:, 12 + s0:15 + s0],
                                        op=OP.subtract)
                nc.scalar.activation(w[:, 12 + s0:15 + s0], var, AF.Sqrt,
                                     bias=eps_col[:])
                nc.vector.reciprocal_approx_fast(var, w[:, 12 + s0:15 + s0])
                nc.vector.tensor_tensor(sc, var,
                                        cvec[:, gbase + s0:gbase + s0 + 3],
                                        op=OP.mult)
                nc.vector.tensor_tensor(sh, mu, sc, op=OP.mult)
                nc.vector.tensor_tensor(sh,
                                        cvec[:, bbase + s0:bbase + s0 + 3],
                                        sh, op=OP.subtract)
                return w

            def wscale(wt, cols):
                """wt[:, i, :] *= cols[:, i] in place; split across engines."""
                nin = wt.shape[1]
                for i in range(nin):
                    ap = wt[:, i, :]
                    sc = cols[:, i:i + 1]
                    if i % 3 == 0:
                        nc.vector.tensor_scalar(ap, ap, sc, zero_col[:],
                                                op0=OP.mult, op1=OP.add)
                    elif i % 3 == 1:
                        nc.scalar.activation(ap, ap, AF.Identity,
                                             bias=zero_col[:], scale=sc)
                    else:
                        nc.gpsimd.tensor_scalar(ap, ap, sc, zero_col[:],
                                                op0=OP.mult, op1=OP.add)

            def pool_apply(dst, src, w, jw, nch=1):
                """dst = src*scale + shift on GpSimd (SBUF only)."""
                nc.gpsimd.tensor_scalar(dst, src,
                                        w[:, 18 + jw:19 + jw],
                                        w[:, 24 + jw:25 + jw],
                                        op0=OP.mult, op1=OP.add)

            # ================= layers =================
            xo_cur = xo1
            xq_cur = xq1
            res_final = None
            for layer in range(layers):
                first = layer == 0
                last = layer == layers - 1
                # ---- Q/K/V projections ----
                qt = tr.tile([128, 6, R], BF, tag="q6R", bufs=1)
                kt = tr.tile([128, 6, S], BF, tag="k6S", bufs=1)
                dense_R(w_sb["wq"], 2, lambda i: xq_cur[:, i, :],
                        lambda j, ps: nc.scalar.activation(
                            qt[:, j, :], ps[:, 0:R], AF.Relu,
                            bias=cvec[:, 0 + j:1 + j]))
                dense_S(w_sb["wk"], 2, lambda i, c0, cw: xin[:, 2 + i, c0:c0 + cw],
                        lambda j, c0, ps: nc.scalar.activation(
                            kt[:, j, c0:c0 + 512], ps[:, 0:512], AF.Relu,
                            bias=cvec[:, 6 + j:7 + j]))
                vt = tr.tile([128, 8, 780], BF, tag="v780", bufs=1)
                vtok(w_sb["wv"],
                     lambda i, tch: xin[:, 4 + i, tch * 128:(tch + 1) * 128],
                     S, vt, relu=True)
                if first:
                    # cross-attn Q2/K2: emitted after L1 QKV so the PE can
                    # start on Q immediately (enc/wq2/wk2 loads are later in
                    # the DMA order than wq/xq/xin).
                    q2 = tr.tile([128, 6, R], BF, tag="q2", bufs=1)
                    k2 = tr.tile([128, 6, S], BF, tag="k2", bufs=1)
                    dense_R(w_sb["wq2"], 3, lambda i: encq[:, i, :],
                            lambda j, ps: nc.vector.tensor_scalar(
                                q2[:, j, :], ps[:, 0:R],
                                cvec[:, 12 + j:13 + j], None, op0=OP.add))
                    dense_S(w_sb["wk2"], 3,
                            lambda i, c0, cw: enck[:, i, c0:c0 + cw],
                            lambda j, c0, ps: nc.vector.tensor_scalar(
                                k2[:, j, c0:c0 + 512], ps[:, 0:512],
                                cvec[:, 18 + j:19 + j], None, op0=OP.add))

                # ---- self attention -> res (x1 = norm(AV) + xo), stats,
                #      raw-x1 bf16 staging chunks for the pair AllGather ----
                res = resp.tile([128, 6, R], F32, tag="res")
                stats1 = smallp.tile([128, 12], F32, tag=f"st{layer}a")
                stats_a = stats1[:, 0:6]
                stats_b = stats1[:, 6:12]
                w1 = smallp.tile([128, 30], F32, tag="bnw", bufs=2,
                                 name="w1")
                tbf = tr.tile([128, 6, S], BF, tag="tbfS", bufs=1)
                x1g = tr.tile([128, 6, R], BF, tag="xg", bufs=2)
                GSZ = (4, 1, 1)
                GOF = (0, 4, 5)
                ag1in = [dram.tile([128, GSZ[g], R], BF,
                                   tag=f"ag1i{layer}g{g}",
                                   name=f"ag1i{g}") for g in range(3)]
                ag1out = [dram.tile([2, 128, GSZ[g], R], BF,
                                    tag=f"ag1o{layer}g{g}",
                                    name=f"ag1o{g}") for g in range(3)]

                ar1box = []

                def self_out(jh, poA, poB, bc, res=res,
                             stats_ab=(stats_a, stats_b), stats1=stats1,
                             x1g=x1g, ag1in=ag1in, ag1out=ag1out, tbf=tbf,
                             w1=w1, ar1box=ar1box):
                    nc.vector.tensor_tensor(res[0:64, jh, :], poA[0:64, 0:R],
                                            bc[0:64, 0:R], op=OP.mult)
                    nc.vector.tensor_tensor(res[64:128, jh, :], poB[0:64, 0:R],
                                            bc[64:128, 0:R], op=OP.mult)
                    nc.vector.tensor_tensor(res[:, jh, :], res[:, jh, :],
                                            xo_cur[:, jh, :], op=OP.add)
                    bn_stats_chunk(res, stats_ab, jh)
                    if jh == 2:
                        # first stats half -> DRAM early (hidden)
                        pass
                    if jh == 5:
                        # AllReduce first on the serial CC channel: its
                        # result gates everything, the last gather only V2.
                        ar1box.append(bn_start(stats1))
                    g = 0 if jh < 4 else (1 if jh < 5 else 2)
                    nc.vector.tensor_copy(x1g[:, jh, :], res[:, jh, :])
                    nc.sync.dma_start(ag1in[g][:, jh - GOF[g], :],
                                      x1g[:, jh, :])
                    if jh - GOF[g] == GSZ[g] - 1:
                        nc.gpsimd.collective_compute(
                            "AllGather", OP.bypass, replica_groups=PAIRS,
                            ins=[ag1in[g][:].opt()], outs=[ag1out[g][:].opt()])
                        nc.sync.dma_start(
                            tbf[:, GOF[g]:GOF[g] + GSZ[g], 0:R],
                            ag1out[g][0, :, :, :])
                        nc.sync.dma_start(
                            tbf[:, GOF[g]:GOF[g] + GSZ[g], R:S],
                            ag1out[g][1, :, :, :])

                po_pair = [None, None]
                rcp_s = None
                for h in range(H):
                    e = epool.tile([128, 8, 512], BF, tag="e8", bufs=3)
                    attn_head_scores(
                        h, qt[64 * (h % 2):64 * (h % 2) + 64, h // 2, :],
                        lambda j, h=h: kt[64 * (h % 2):64 * (h % 2) + 64,
                                          h // 2, j * 128:(j + 1) * 128],
                        e, "square", SCALE1)
                    po_pair[h % 2] = attn_head_av(h, vt, e)
                    if h % 2 == 0:
                        rcp_s = smallp.tile([1, 2, R], BF, tag="rcps", bufs=2)
                    attn_head_den(po_pair[h % 2], rcp_s, h % 2)
                    if h % 2 == 1:
                        attn_pair_finish(h // 2, po_pair[0], po_pair[1],
                                         rcp_s[0:1, :, :], self_out)

                arout1 = ar1box[0]
                bn_w_half(arout1, 0, w1, 24, 30)
                bn_w_half(arout1, 1, w1, 24, 30)
                if stage <= 1:
                    res_final = res
                    break

                # ---- window filler: first cross-score heads (L1) /
                #      e + rcp prefetch from DRAM (L2) ----
                e_held = {}

                def cross_e(h):
                    e = epool.tile([128, 8, 512], BF, tag="e8", bufs=3)
                    if first:
                        attn_head_scores(
                            h, q2[64 * (h % 2):64 * (h % 2) + 64, h // 2, :],
                            lambda j, h=h: k2[64 * (h % 2):64 * (h % 2) + 64,
                                              h // 2, j * 128:(j + 1) * 128],
                            e, "exp", SCALE2)
                        nc.scalar.dma_start(a2d[:, :, h * 512:(h + 1) * 512],
                                            e[:])
                    else:
                        nc.scalar.dma_start(e[:],
                                            a2d[:, :, h * 512:(h + 1) * 512])
                    return e

                NPRE = 3
                for h in range(NPRE):
                    e_held[h] = cross_e(h)

                # fold BN1's scale into Wv2 (diag(s1)·Wv2): V2 then runs on
                # RAW gathered x1 — no BN applies on the critical path. The
                # BN shift contributes a constant row to V2 that the next
                # train-mode BN cancels exactly. L2 rescales by the ratio
                # s1_L2/s1_L1 (wv2 is modified in place).
                if first:
                    nc.vector.tensor_copy(s1sav[:], w1[:, 18:24])
                    wv2cols = w1[:, 18:24]
                else:
                    rat1 = smallp.tile([128, 6], F32, tag="rat", bufs=2,
                                       name="rat1")
                    nc.vector.reciprocal_approx_fast(rat1[:], s1sav[:])
                    nc.vector.tensor_tensor(rat1[:], rat1[:], w1[:, 18:24],
                                            op=OP.mult)
                    wv2cols = rat1[:]
                wscale(w_sb["wv2"], wv2cols)
                if stage <= 2:
                    res_final = res
                    break

                # ---- V2 for ALL tokens (no V2 collective) ----
                v2 = tr.tile([128, 8, 780], BF, tag="v780", bufs=1)
                vtok(w_sb["wv2"],
                     lambda i, tch: tbf[:, i, tch * 128:(tch + 1) * 128],
                     S, v2, relu=False)
                # t residual (f32, own rows) for x2 = m2@Wo2 + t
                for j in range(6):
                    pool_apply(res[:, j, :], res[:, j, :], w1, j)

                # ---- cross attention AV (+ remaining scores/loads) -> m2 ----
                m2 = tr.tile([128, 6, R], BF, tag="q6R", bufs=1)

                def cross_out(jh, poA, poB, bc, m2=m2):
                    nc.vector.tensor_tensor(m2[0:64, jh, :], poA[0:64, 0:R],
                                            bc[0:64, 0:R], op=OP.mult)
                    nc.vector.tensor_tensor(m2[64:128, jh, :], poB[0:64, 0:R],
                                            bc[64:128, 0:R], op=OP.mult)

                if not last:
                    # gather m2 chunk-groups during cross-AV: Wo2 is then
                    # recomputed for BOTH halves locally (x2 = m2g@Wo2 + t),
                    # so BN2 and BN3 expose only their AllReduce.
                    m2g_t = tr.tile([128, 8, 780], BF, tag="v780", bufs=1)
                    m2g = m2g_t.rearrange("p a b -> p (a b)")[:, 0:6 * S]
                    m2g = m2g.rearrange("p (c s) -> p c s", c=6)
                    agm_in = [dram.tile([128, GSZ[g], R], BF,
                                        tag=f"agm{layer}g{g}",
                                        name=f"agmi{g}") for g in range(3)]
                    agm_out = [dram.tile([2, 128, GSZ[g], R], BF,
                                         tag=f"agmo{layer}g{g}",
                                         name=f"agmo{g}") for g in range(3)]

                po_pair = [None, None]
                rcp_p = None
                for h in range(H):
                    e = e_held.pop(h) if h in e_held else cross_e(h)
                    if h + NPRE < H:
                        e_held[h + NPRE] = cross_e(h + NPRE)
                    po_pair[h % 2] = attn_head_av(h, v2, e)
                    if h % 2 == 0:
                        rcp_p = smallp.tile([1, 2, R], BF, tag="rcps",
                                            bufs=2, name="rcp_p")
                        if not first:
                            nc.sync.dma_start(rcp_p[:],
                                              rcpd[0:1, h:h + 2, :])
                    if first:
                        attn_head_den(po_pair[h % 2], rcp_p, h % 2)
                    if h % 2 == 1:
                        attn_pair_finish(h // 2, po_pair[0], po_pair[1],
                                         rcp_p[0:1, :, :], cross_out)
                        if first:
                            nc.sync.dma_start(rcpd[0:1, h - 1:h + 1, :],
                                              rcp_p[:])
                        if not last:
                            jh = h // 2
                            g = 0 if jh < 4 else (1 if jh < 5 else 2)
                            nc.sync.dma_start(agm_in[g][:, jh - GOF[g], :],
                                              m2[:, jh, :])
                            if jh - GOF[g] == GSZ[g] - 1:
                                nc.gpsimd.collective_compute(
                                    "AllGather", OP.bypass,
                                    replica_groups=PAIRS,
                                    ins=[agm_in[g][:].opt()],
                                    outs=[agm_out[g][:].opt()])
                                nc.sync.dma_start(
                                    m2g[:, GOF[g]:GOF[g] + GSZ[g], 0:R],
                                    agm_out[g][0, :, :, :])
                                nc.sync.dma_start(
                                    m2g[:, GOF[g]:GOF[g] + GSZ[g], R:S],
                                    agm_out[g][1, :, :, :])
                if stage <= 3:
                    res_final = res
                    break

                # ---- x2 = m2 @ Wo2 + t ; stats2 (bo2 dropped: BN removes) ----
                res2 = resp.tile([128, 6, R], F32, tag="res")
                stats2 = smallp.tile([128, 12], F32, tag=f"st{layer}b")
                st2a = stats2[:, 0:6]
                st2b = stats2[:, 6:12]
                w2 = smallp.tile([128, 30], F32, tag="bnw", bufs=2,
                                 name="w2")
                t_prev = res
                ar2box = []

                x2bf = tr.tile([128, 6, R], BF, tag="q2", bufs=1,
                               name="x2bf")

                def wo2_evict(j, ps, res2=res2, stats_ab=(st2a, st2b),
                              t_prev=t_prev, w2=w2, stats2=stats2,
                              ar2box=ar2box, x2bf=x2bf):
                    nc.vector.tensor_tensor(res2[:, j, :], ps[:, 0:R],
                                            t_prev[:, j, :], op=OP.add)
                    bn_stats_chunk(res2, stats_ab, j)
                    nc.scalar.copy(x2bf[:, j, :], res2[:, j, :])
                    if j == 5:
                        ar2box.append(bn_start(stats2))

                dense_R(w_sb["wo2"], 6, lambda i: m2[:, i, :], wo2_evict)
                if not last:
                    # both-halves Wo2 from gathered m2; evict x2 = ps + t
                    # (tbf) into t2g raw — PE work that hides the AR2 wait.
                    t2g = tr.tile([128, 6, S], BF, tag="k2", bufs=1,
                                  name="t2g")
                    for j in range(6):
                        ps = ps_sc.tile([128, 2, 512], F32, tag="psc")
                        for i in range(6):
                            for ci in range(2):
                                nc.tensor.matmul(
                                    ps[:, ci, :],
                                    w_sb["wo2"][:, i, j * 128:(j + 1) * 128],
                                    m2g[:, i, ci * 512:ci * 512 + 512],
                                    start=(i == 0), stop=(i == 5))
                        for ci in range(2):
                            nc.vector.affine_then_add(
                                t2g[:, j, ci * 512:ci * 512 + 512],
                                tbf[:, j, ci * 512:ci * 512 + 512],
                                ps[:, ci, :],
                                w1[:, 18 + j:19 + j], w1[:, 24 + j:25 + j])
                arout2 = ar2box[0]
                bn_w_half(arout2, 0, w2, 36, 42)
                bn_w_half(arout2, 1, w2, 36, 42)   # g2, b2
                # t2 bf16 (FFN moving operand) on Act; res2 in-place f32
                # (x3 residual) on Pool.
                # fold BN2's scale into Wf: the FFN matmul consumes the
                # raw x2 (cast during Wo2); constant shifts cancel in BN3.
                if first:
                    nc.vector.tensor_copy(s2sav[:], w2[:, 18:24])
                    wfcols = w2[:, 18:24]
                else:
                    rat2 = smallp.tile([128, 6], F32, tag="rat", bufs=2,
                                       name="rat2")
                    nc.vector.reciprocal_approx_fast(rat2[:], s2sav[:])
                    nc.vector.tensor_tensor(rat2[:], rat2[:], w2[:, 18:24],
                                            op=OP.mult)
                    wfcols = rat2[:]
                wscale(w_sb["wf"], wfcols)
                for j in range(6):
                    pool_apply(res2[:, j, :], res2[:, j, :], w2, j)
                if stage <= 4:
                    res_final = res2
                    break

                # ---- FFN: x3 = t2 @ Wf + t2 ; stats3 (bf dropped) ----
                res3 = resp.tile([128, 6, R], F32, tag="res")
                stats3 = smallp.tile([128, 12], F32, tag=f"st{layer}c")
                st3a = stats3[:, 0:6]
                st3b = stats3[:, 6:12]
                w3 = smallp.tile([128, 30], F32, tag="bnw", bufs=2,
                                 name="w3")
                ar3box = []

                def wf_evict(j, ps, res3=res3, stats_ab=(st3a, st3b),
                             stats3=stats3, res2=res2, w3=w3, ar3box=ar3box):
                    nc.vector.tensor_tensor(res3[:, j, :], ps[:, 0:R],
                                            res2[:, j, :], op=OP.add)
                    bn_stats_chunk(res3, stats_ab, j)
                    if j == 5:
                        ar3box.append(bn_start(stats3))

                dense_R(w_sb["wf"], 6, lambda i: x2bf[:, i, :], wf_evict)
                if not last:
                    # recompute the FFN for ALL tokens from gathered t2 for
                    # the xin chunks L2 actually reads (K: 2,3 / V: 4,5) —
                    # this PE work fills the AR3 window.
                    for j in range(2, 6):
                        ps = ps_sc.tile([128, 2, 512], F32, tag="psc")
                        for i in range(6):
                            for ci in range(2):
                                nc.tensor.matmul(
                                    ps[:, ci, :],
                                    w_sb["wf"][:, i, j * 128:(j + 1) * 128],
                                    t2g[:, i, ci * 512:ci * 512 + 512],
                                    start=(i == 0), stop=(i == 5))
                        for ci in range(2):
                            nc.vector.affine_then_add(
                                xin[:, j, ci * 512:ci * 512 + 512],
                                t2g[:, j, ci * 512:ci * 512 + 512],
                                ps[:, ci, :],
                                w2[:, 18 + j:19 + j], w2[:, 24 + j:25 + j])
                arout3 = ar3box[0]
                bn_w_half(arout3, 0, w3, 36, 42)
                bn_w_half(arout3, 1, w3, 36, 42)   # g2, b2 (FFN BN)

                if not last:
                    # local-first: xq (Q-L2's input) is derivable from res3
                    # alone, so Q can run while the gathers land. res3 in
                    # place f32 -> xo (Pool); xq bf16 via Act.
                    xq2 = tr.tile([128, 2, R], BF, tag="xq", bufs=1)
                    for j in range(2):
                        nc.vector.tensor_scalar(
                            xq2[:, j, :], res3[:, j, :],
                            w3[:, 18 + j:19 + j], w3[:, 24 + j:25 + j],
                            op0=OP.mult, op1=OP.add)
                    for j in range(2, 6):
                        for half in range(2):
                            ap = xin[:, j, half * R:(half + 1) * R]
                            eng = (2 * j + half) % 3
                            if eng == 0:
                                pool_apply(ap, ap, w3, j)
                            elif eng == 1:
                                nc.scalar.activation(
                                    ap, ap, AF.Identity,
                                    bias=w3[:, 24 + j:25 + j],
                                    scale=w3[:, 18 + j:19 + j])
                            else:
                                nc.vector.tensor_scalar(
                                    ap, ap, w3[:, 18 + j:19 + j],
                                    w3[:, 24 + j:25 + j],
                                    op0=OP.mult, op1=OP.add)
                    for j in range(6):
                        pool_apply(res3[:, j, :], res3[:, j, :], w3, j)
                    xo_cur = res3
                    xq_cur = xq2
                else:
                    # final: BN apply (bf16 out staging) + chunk DMAs split
                    # over the two HWDGE queues
                    obf = tr.tile([128, 6, R], BF, tag="xg", bufs=2,
                                  name="obf")
                    for j in range(6):
                        if j % 3 == 0:
                            nc.gpsimd.tensor_scalar(
                                obf[:, j, :], res3[:, j, :],
                                w3[:, 18 + j:19 + j], w3[:, 24 + j:25 + j],
                                op0=OP.mult, op1=OP.add)
                        elif j % 3 == 1:
                            nc.scalar.activation(obf[:, j, :], res3[:, j, :],
                                                 AF.Identity,
                                                 bias=w3[:, 24 + j:25 + j],
                                                 scale=w3[:, 18 + j:19 + j])
                        else:
                            nc.vector.tensor_scalar(
                                obf[:, j, :], res3[:, j, :],
                                w3[:, 18 + j:19 + j], w3[:, 24 + j:25 + j],
                                op0=OP.mult, op1=OP.add)
                        eng = nc.sync if j % 2 == 0 else nc.scalar
                        eng.dma_start(out_io[:, j, :], obf[:, j, :])
                    res_final = None

            if res_final is not None:
                nc.sync.dma_start(out_io[:], res_final[:])

    nc.compile()
    return nc


def _host_prepare(inputs):
    x = np.asarray(inputs["x"])
    encod = np.asarray(inputs["encod"], np.float32)
    embed = np.asarray(inputs["embed"], np.float32)
    emb = embed[x.astype(np.int64)]
    im0 = 2.0 * emb + _pos_encoding()[None]  # [B,S,D] f32

    wq, wk, wv = (np.asarray(inputs[k], np.float32) for k in ("Wq", "Wk", "Wv"))
    wq2, wk2 = (np.asarray(inputs[k], np.float32) for k in ("Wq2", "Wk2"))
    wv2, wo2, wf = (np.asarray(inputs[k], np.float32) for k in ("Wv2", "Wo2", "Wf"))
    w_np = {nm: _bf16(_wchunk(w)) for nm, w in
            [("wq", wq), ("wk", wk), ("wv", wv), ("wq2", wq2), ("wk2", wk2),
             ("wv2", wv2), ("wo2", wo2), ("wf", wf)]}
    cvec = np.concatenate(
        [_col(np.asarray(inputs[k], np.float32)) for k in
         ("bq", "bk", "bq2", "bk2", "g1", "b1", "g2", "b2")],
        axis=1).astype(np.float32)
    brow = _bf16(np.asarray(inputs["bv"], np.float32)[None, :])

    in_maps = []
    for c in range(NC):
        b_, r_ = c // 2, c % 2
        rows = slice(r_ * R, (r_ + 1) * R)
        m = dict(w_np)
        m["cvec"] = cvec
        m["brow"] = brow
        m["xin"] = _bf16(_fm(im0[b_]))
        m["xq"] = _bf16(_fm(im0[b_][rows, 0:256]))
        m["xo"] = _fm(im0[b_][rows]).astype(np.float32)
        m["encq"] = _bf16(_fm(encod[b_][rows, 0:384]))
        m["enck"] = _bf16(_fm(encod[b_][:, 384:768]))
        in_maps.append(m)
    return in_maps


def _gather(results):
    out = np.zeros((B, S, D), np.float32)
    for c in range(NC):
        b_, r_ = c // 2, c % 2
        a = np.asarray(results[c]["out"], np.float32)  # [128, 6, R] bf16
        out[b_, r_ * R:(r_ + 1) * R] = a.transpose(1, 0, 2).reshape(D, R).T
    return out


def kernel(**inputs) -> np.ndarray:
    from concourse.bass_utils import run_bass_kernel_spmd

    if "nc" not in _CACHE:
        _CACHE["nc"] = _build()
    nc = _CACHE["nc"]
    in_maps = _host_prepare(inputs)
    res = run_bass_kernel_spmd(nc, in_maps, core_ids=list(range(NC)))
    return _gather(res.results)
